# revision 10
# baseline (speedup 1.0000x reference)
"""GCN link-prediction kernel for Trainium2 (8 NeuronCores, SPMD).

  h = relu(D^-1/2 (A+I) D^-1/2 (x@W1) + b1)     deg = in-degree + 1
  z = D^-1/2 (A+I) D^-1/2 (h@W2) + b2
  logits[k] = z[e0_k] . z[e1_k]   over pos+neg eval edges

Sharding: nodes -> degree-sorted windows of 128 lanes, window w -> core w%8.
Per-layer tables (w = dinv*(x@W1), v = [u@W2|0], z = [z|0]) are AllGathered to
DRAM [NPOS, 64]; per-edge messages are fetched with the GPSIMD dma_gather
extended instruction (int16 indices -> 4 address regions of 32768 rows; edge
lists laid out region-major inside window chunks; 4 SWDGE queues). Segment
sums are lane-aligned strided DVE reduces with per-(chunk,region) uniform K;
nodes are reordered within (degree x region) groups by per-region in-edge
count tuples so K ~= mean. Self-loop terms are applied as local elementwise
ops. Decode groups eval edges by (region(e0), region(e1)).
"""

import os
import numpy as np

_TINY = bool(os.environ.get("GNN_TINY"))

if _TINY:
    N_NODES, N_EDGES, N_EVAL = 2000, 32768, 2048
    WPC, REG = 2, 512
else:
    N_NODES, N_EDGES, N_EVAL = 100000, 1600000, 100000
    WPC, REG = 98, 32768

F_IN, H1, H2 = 256, 64, 32
NCORES = 8
PC = WPC * 128
NPOS = NCORES * PC
NREG = 4
REGION_ROWS = [REG, REG, REG, NPOS - 3 * REG]
EV_PER_CORE = 2 * N_EVAL // NCORES
SLOT_CAP = 96           # msg-tile slots per chunk (pairs of windows)
MAXW = 2                 # max windows per chunk (for tile sizing)


def _region_of_pos(p):
    return np.minimum(np.asarray(p) // REG, NREG - 1)


# ---------------------------------------------------------------- host prep
def _preprocess(edge_index, pos_edge_index, neg_edge_index):
    src = np.asarray(edge_index[0], dtype=np.int64)
    dst = np.asarray(edge_index[1], dtype=np.int64)
    deg = np.bincount(dst, minlength=N_NODES) + 1
    dinv = (1.0 / np.sqrt(deg)).astype(np.float32)

    # rank nodes by degree; rank -> (core, j, lane) -> position
    order = np.argsort(-deg, kind="stable")
    ranks = np.arange(NPOS)
    w_of_rank = ranks // 128
    pos_of_rank = ((w_of_rank % NCORES) * PC + (w_of_rank // NCORES) * 128
                   + ranks % 128)
    region_of_rank = _region_of_pos(pos_of_rank)

    node_rank0 = np.full(N_NODES, -1, np.int64)
    node_rank0[order] = np.arange(N_NODES)
    node_region = region_of_rank[node_rank0]

    # per-dst source-region tuples
    tup = np.zeros((N_NODES, NREG), np.int64)
    np.add.at(tup, (dst, node_region[src]), 1)
    tup_key = ((tup[:, 0] * 128 + tup[:, 1]) * 128 + tup[:, 2])

    # reorder nodes within (degree, slot-region) groups by tuple
    slot_deg = deg[order]
    grp = slot_deg * NREG + region_of_rank[:N_NODES]
    grun = np.argsort(grp, kind="stable")
    gsorted = grp[grun]
    starts = np.concatenate([[0], np.where(gsorted[1:] != gsorted[:-1])[0] + 1,
                             [N_NODES]])
    final = order.copy()
    for a, b in zip(starts[:-1], starts[1:]):
        slots = grun[a:b]
        nodes = order[slots]
        nodes = nodes[np.argsort(tup_key[nodes], kind="stable")]
        final[np.sort(slots)] = nodes
    order = final
    node_rank = np.full(N_NODES, -1, np.int64)
    node_rank[order] = np.arange(N_NODES)
    node_pos = pos_of_rank[node_rank]

    # reserve one zero row per region
    zero_pos = [REG - 1, 2 * REG - 1, 3 * REG - 1, NPOS - 1]
    pos_node = np.full(NPOS, -1, np.int64)
    pos_node[node_pos] = np.arange(N_NODES)
    free_pos = np.where(pos_node < 0)[0]
    free_pos = free_pos[~np.isin(free_pos, zero_pos)]
    fi = 0
    for zp in zero_pos:
        occ = pos_node[zp]
        if occ >= 0:
            npp = free_pos[fi]; fi += 1
            pos_node[zp] = -1
            pos_node[npp] = occ
            node_pos[occ] = npp
    node_region = _region_of_pos(node_pos)

    core_of = node_pos // PC
    j_of = (node_pos % PC) // 128
    lane_of = node_pos % 128

    # per (core, j, lane, region) counts -> shared K[j][r]
    cnt = np.zeros((NCORES, WPC, 128, NREG), np.int64)
    np.add.at(cnt, (core_of[dst], j_of[dst], lane_of[dst], node_region[src]), 1)
    Kjr = cnt.max(axis=(0, 2))                  # [WPC, NREG]

    # chunks: even window counts, slots <= cap
    chunks = []   # (j0, nw)
    j0 = 0
    while j0 < WPC:
        nw = 2
        while (j0 + nw + 2 <= WPC and nw + 2 <= MAXW
               and (Kjr[j0:j0 + nw + 2].max(axis=0).sum() * (nw + 2)) <= SLOT_CAP):
            nw += 2
        chunks.append((j0, nw))
        j0 += nw
    NCH = len(chunks)
    Kcr = np.zeros((NCH, NREG), np.int64)
    for ci, (j0, nw) in enumerate(chunks):
        Kcr[ci] = Kjr[j0:j0 + nw].max(axis=0)
    chunk_slots = np.array([Kcr[ci].sum() * nw for ci, (_, nw) in enumerate(chunks)])
    S_TOTAL = int(chunk_slots.sum())
    chunk_base = np.concatenate([[0], np.cumsum(chunk_slots)])[:-1]
    roff = np.zeros((NCH, NREG), np.int64)
    for ci, (_, nw) in enumerate(chunks):
        roff[ci] = np.concatenate([[0], np.cumsum(Kcr[ci] * nw)])[:-1]

    # per-edge slot assignment
    ekey = ((core_of[dst] * WPC + j_of[dst]) * 128 + lane_of[dst]) * NREG \
        + node_region[src]
    eord = np.argsort(ekey, kind="stable")
    ks = ekey[eord]
    first = np.concatenate([[True], ks[1:] != ks[:-1]])
    gstart = np.where(first)[0]
    gid = np.cumsum(first) - 1
    krank = np.arange(len(ks)) - gstart[gid]
    ci_of_j = np.zeros(WPC, np.int64)
    jin_of_j = np.zeros(WPC, np.int64)
    for ci, (j0, nw) in enumerate(chunks):
        ci_of_j[j0:j0 + nw] = ci
        jin_of_j[j0:j0 + nw] = np.arange(nw)

    d_, s_ = dst[eord], src[eord]
    ecore, ej, el = core_of[d_], j_of[d_], lane_of[d_]
    er = node_region[s_]
    eci = ci_of_j[ej]
    slot = chunk_base[eci] + roff[eci, er] + jin_of_j[ej] * Kcr[eci, er] + krank
    lpos = slot * 128 + el

    reb_zero = [zero_pos[r] - r * REG for r in range(NREG)]
    flat = np.empty((NCORES, S_TOTAL * 128), np.int64)
    for ci, (j0, nw) in enumerate(chunks):
        for r in range(NREG):
            a = int((chunk_base[ci] + roff[ci, r]) * 128)
            b = int(a + nw * Kcr[ci, r] * 128)
            flat[:, a:b] = reb_zero[r]
    flat[ecore, lpos] = node_pos[s_] - er * REG
    idx16 = np.tile(flat.reshape(NCORES, S_TOTAL * 8, 16).transpose(0, 2, 1),
                    (1, 8, 1)).astype(np.int16)

    # eval edges
    ev = np.concatenate([np.asarray(pos_edge_index, np.int64),
                         np.asarray(neg_edge_index, np.int64)], axis=1)
    p0, p1 = node_pos[ev[0]], node_pos[ev[1]]
    r0, r1 = _region_of_pos(p0), _region_of_pos(p1)
    evc = np.arange(2 * N_EVAL) // EV_PER_CORE
    gidx = r0 * NREG + r1
    gcnt = np.zeros((NCORES, 16), np.int64)
    np.add.at(gcnt, (evc, gidx), 1)
    gpad = ((gcnt.max(axis=0) + 127) // 128) * 128
    gslots = gpad // 128
    S_EV = int(gslots.sum())
    gbase = np.concatenate([[0], np.cumsum(gpad)])[:-1]
    ekey2 = evc * 16 + gidx
    eo2 = np.argsort(ekey2, kind="stable")
    ks2 = ekey2[eo2]
    first2 = np.concatenate([[True], ks2[1:] != ks2[:-1]])
    gstart2 = np.where(first2)[0]
    gid2 = np.cumsum(first2) - 1
    rank2 = np.arange(len(ks2)) - gstart2[gid2]
    lpos2 = gbase[gidx[eo2]] + rank2
    ev0 = np.empty((NCORES, S_EV * 128), np.int64)
    ev1 = np.empty((NCORES, S_EV * 128), np.int64)
    for g in range(16):
        a, b = int(gbase[g]), int(gbase[g] + gpad[g])
        ev0[:, a:b] = reb_zero[g // NREG]
        ev1[:, a:b] = reb_zero[g % NREG]
    ev0[evc[eo2], lpos2] = p0[eo2] - r0[eo2] * REG
    ev1[evc[eo2], lpos2] = p1[eo2] - r1[eo2] * REG
    ev0_16 = np.tile(ev0.reshape(NCORES, S_EV * 8, 16).transpose(0, 2, 1),
                     (1, 8, 1)).astype(np.int16)
    ev1_16 = np.tile(ev1.reshape(NCORES, S_EV * 8, 16).transpose(0, 2, 1),
                     (1, 8, 1)).astype(np.int16)

    return dict(
        dinv=dinv, node_pos=node_pos, chunks=chunks, Kcr=Kcr,
        chunk_base=chunk_base, chunk_slots=chunk_slots, roff=roff,
        S_TOTAL=S_TOTAL, idx16=idx16, gslots=gslots, S_EV=S_EV,
        ev0_16=ev0_16, ev1_16=ev1_16,
        ev_core=evc[eo2], ev_lpos=lpos2, ev_edge=eo2,
    )


def _stage_inputs(x, W1, b1, W2, b2, meta):
    dinv, node_pos = meta["dinv"], meta["node_pos"]
    xT = np.zeros((NCORES, 128, 2, PC), np.float32)
    dv = np.zeros((NCORES, 128, WPC), np.float32)
    core_of = node_pos // PC
    loc = node_pos % PC
    xv = np.asarray(x, np.float32).reshape(N_NODES, 2, 128).transpose(0, 2, 1)
    xT[core_of, :, :, loc] = xv
    dv[core_of, loc % 128, loc // 128] = dinv
    dinv64 = np.repeat(dv[:, :, :, None], H1, axis=3).reshape(NCORES, 128, WPC * H1)
    dinv32 = np.repeat(dv[:, :, :, None], H2, axis=3).reshape(NCORES, 128, WPC * H2)
    W1r = np.asarray(W1, np.float32).reshape(2, 128, H1).transpose(1, 0, 2).copy()
    W2s = np.zeros((128, 128), np.float32)
    W2a = np.asarray(W2, np.float32)
    W2s[:H1, :H2] = W2a
    W2s[H1:, 2 * H2:3 * H2] = W2a
    b1r = np.tile(np.asarray(b1, np.float32)[None, :], (128, MAXW))
    b2r = np.tile(np.asarray(b2, np.float32)[None, :], (128, MAXW))
    ident = np.eye(128, dtype=np.float32)
    return [{
        "xT": xT[c].reshape(128, 2 * PC),
        "dinv64": dinv64[c], "dinv32": dinv32[c],
        "W1r": W1r.reshape(128, 2 * H1), "W2s": W2s,
        "b1r": b1r, "b2r": b2r, "ident": ident,
        "idx16": meta["idx16"][c],
        "ev0": meta["ev0_16"][c], "ev1": meta["ev1_16"][c],
    } for c in range(NCORES)]


# ---------------------------------------------------------------- bass build
def _build(meta):
    import concourse.bacc as bacc
    import concourse.mybir as mybir
    import concourse.tile as tile

    dt = mybir.dt
    AT = mybir.AxisListType
    OP = mybir.AluOpType

    chunks = meta["chunks"]
    Kcr = meta["Kcr"]
    chunk_base = meta["chunk_base"]
    chunk_slots = meta["chunk_slots"]
    roff = meta["roff"]
    S_TOTAL = int(meta["S_TOTAL"])
    gslots = meta["gslots"]
    S_EV = int(meta["S_EV"])
    MAXSL = int(chunk_slots.max())
    MAXEV = int(gslots.max())

    nc = bacc.Bacc("TRN2", target_bir_lowering=False, debug=False,
                   num_swdge_queues=4)

    ei = lambda n, s: nc.dram_tensor(n, s, dt.float32, kind="ExternalInput")
    xT_d = ei("xT", [128, 2 * PC])
    dinv64_d = ei("dinv64", [128, WPC * H1])
    dinv32_d = ei("dinv32", [128, WPC * H2])
    W1_d = ei("W1r", [128, 2 * H1])
    W2_d = ei("W2s", [128, 128])
    b1_d = ei("b1r", [128, MAXW * H1])
    b2_d = ei("b2r", [128, MAXW * H2])
    id_d = ei("ident", [128, 128])
    idx_d = nc.dram_tensor("idx16", [128, S_TOTAL * 8], dt.int16, kind="ExternalInput")
    ev0_d = nc.dram_tensor("ev0", [128, S_EV * 8], dt.int16, kind="ExternalInput")
    ev1_d = nc.dram_tensor("ev1", [128, S_EV * 8], dt.int16, kind="ExternalInput")
    out_d = nc.dram_tensor("logits", [128, S_EV], dt.float32, kind="ExternalOutput")

    w_in = nc.dram_tensor("w_in", [PC, H1], dt.float32)
    v_in = nc.dram_tensor("v_in", [PC, 2 * H2], dt.float32)
    z_in = nc.dram_tensor("z_in", [PC, 2 * H2], dt.float32)
    w_full = nc.dram_tensor("w_full", [NPOS, H1], dt.float32)
    v_full = nc.dram_tensor("v_full", [NPOS, 2 * H2], dt.float32)
    z_full = nc.dram_tensor("z_full", [NPOS, 2 * H2], dt.float32)
    rg = [list(range(NCORES))]

    qctr = [0]

    def nextq():
        q = qctr[0] % 4
        qctr[0] += 1
        return q

    with tile.TileContext(nc) as tc:
        with (
            tc.tile_pool(name="pers", bufs=1) as pers,
            tc.tile_pool(name="io", bufs=3) as io,
            tc.tile_pool(name="msgp", bufs=3) as msgp,
            tc.tile_pool(name="work", bufs=2) as work,
            tc.tile_pool(name="ps", bufs=2, space="PSUM") as psum,
            tc.tile_pool(name="ps2", bufs=2, space="PSUM") as psum2,
        ):
            W1_sb = pers.tile([128, 2, H1], dt.float32)
            nc.sync.dma_start(W1_sb[:, :, :],
                              W1_d[:, :].rearrange("p (k h) -> p k h", k=2))
            W2_sb = pers.tile([128, 128], dt.float32)
            nc.sync.dma_start(W2_sb[:, :], W2_d[:, :])
            b1_sb = pers.tile([128, MAXW * H1], dt.float32)
            nc.sync.dma_start(b1_sb[:, :], b1_d[:, :])
            b2_sb = pers.tile([128, MAXW * H2], dt.float32)
            nc.sync.dma_start(b2_sb[:, :], b2_d[:, :])
            id_sb = pers.tile([128, 128], dt.float32)
            nc.sync.dma_start(id_sb[:, :], id_d[:, :])
            zbias = pers.tile([128, 1], dt.float32)
            nc.vector.memset(zbias[:, :], 0.0)
            w_loc = pers.tile([128, WPC, H1], dt.float32)
            v_loc = pers.tile([128, WPC, 2 * H2], dt.float32)

            # ---- phase 1: w = dinv * (x @ W1)
            XW = 4
            for c0 in range(0, WPC, XW):
                nw = min(XW, WPC - c0)
                xt = io.tile([128, 2, XW * 128], dt.float32, tag="xt")
                nc.sync.dma_start(
                    xt[:, :, :nw * 128],
                    xT_d[:, :].rearrange("p (k q) -> p k q", k=2)
                    [:, :, c0 * 128:(c0 + nw) * 128])
                dvt = io.tile([128, XW * H1], dt.float32, tag="dv1")
                nc.sync.dma_start(dvt[:, :nw * H1],
                                  dinv64_d[:, c0 * H1:(c0 + nw) * H1])
                for jj in range(nw):
                    hp = psum.tile([128, H1], dt.float32, tag="hps")
                    for k in range(2):
                        nc.tensor.matmul(
                            hp[:, :], xt[:, k, jj * 128:(jj + 1) * 128],
                            W1_sb[:, k, :], start=(k == 0), stop=(k == 1))
                    nc.vector.tensor_tensor(
                        w_loc[:, c0 + jj, :], hp[:, :],
                        dvt[:, jj * H1:(jj + 1) * H1], op=OP.mult)
                nc.sync.dma_start(
                    w_in[:, :].rearrange("(j l) h -> l j h", l=128)[:, c0:c0 + nw, :],
                    w_loc[:, c0:c0 + nw, :])

            nc.gpsimd.collective_compute(
                "AllGather", OP.bypass, replica_groups=rg,
                ins=[w_in.ap().opt()], outs=[w_full.ap().opt()])

            # ---- shared message-pass layer
            def layer(table, H, dinv_d, b_sb, relu, loc_tile, out_wide, wout):
                for ci, (j0, nw) in enumerate(chunks):
                    nsl = int(chunk_slots[ci])
                    cb = int(chunk_base[ci])
                    idxt = io.tile([128, 8 * MAXSL], dt.int16, tag="idxc")
                    nc.sync.dma_start(idxt[:, :nsl * 8],
                                      idx_d[:, cb * 8:(cb + nsl) * 8])
                    dvt = io.tile([128, MAXW * H], dt.float32, tag=f"dv{H}")
                    nc.sync.dma_start(dvt[:, :nw * H],
                                      dinv_d[:, j0 * H:(j0 + nw) * H])
                    msg = msgp.tile([128, MAXSL, 64], dt.float32, tag="msg")
                    terms = []
                    for r in range(NREG):
                        k = int(Kcr[ci, r])
                        if k == 0:
                            continue
                        s0 = int(roff[ci, r])
                        tot = nw * k
                        for t0 in range(0, tot, 8):
                            ts = min(8, tot - t0)
                            n = ts * 128
                            a = s0 + t0
                            nc.gpsimd.dma_gather(
                                out_ap=msg[:, a:a + ts, :],
                                in_ap=table[r * REG:r * REG + REGION_ROWS[r], :],
                                idxs_ap=idxt[:, a * 8:(a + ts) * 8],
                                num_idxs=n, num_idxs_reg=n, elem_size=64,
                                single_packet=True, queue_num=nextq())
                        terms.append((r, k, s0))
                    acc = work.tile([128, MAXW, H], dt.float32, tag=f"acc{H}")
                    rs = work.tile([128, NREG, MAXW, H], dt.float32, tag=f"rs{H}")
                    for (r, k, s0) in terms:
                        view = msg[:, s0:s0 + nw * k, :H].rearrange(
                            "p (w k) h -> p w h k", k=k)
                        nc.vector.tensor_reduce(
                            rs[:, r, :nw, :], view, axis=AT.X, op=OP.add)
                    # acc = sum(rs terms) + self rows
                    nc.vector.tensor_tensor(
                        acc[:, :nw, :], rs[:, terms[0][0], :nw, :],
                        loc_tile[:, j0:j0 + nw, :H], op=OP.add)
                    for (r, _, _) in terms[1:]:
                        nc.vector.tensor_tensor(
                            acc[:, :nw, :], acc[:, :nw, :], rs[:, r, :nw, :],
                            op=OP.add)
                    dvv = dvt[:, :nw * H].rearrange("p (w h) -> p w h", h=H)
                    nc.vector.tensor_tensor(acc[:, :nw, :], acc[:, :nw, :],
                                            dvv, op=OP.mult)
                    bb = b_sb[:, :nw * H].rearrange("p (w h) -> p w h", h=H)
                    nc.vector.tensor_tensor(acc[:, :nw, :], acc[:, :nw, :],
                                            bb, op=OP.add)
                    if relu:
                        nc.scalar.activation(
                            acc[:, :nw, :], acc[:, :nw, :],
                            mybir.ActivationFunctionType.Relu, bias=zbias[:, :])
                        nc.vector.tensor_tensor(acc[:, :nw, :], acc[:, :nw, :],
                                                dvv, op=OP.mult)
                        for jj in range(0, nw, 2):
                            tp = psum.tile([128, 128], dt.float32, tag="tp")
                            nc.tensor.transpose(
                                tp[:, :],
                                acc[:, jj:jj + 2, :].rearrange("p w h -> p (w h)"),
                                id_sb[:, :])
                            ut = work.tile([128, 128], dt.float32, tag="ut")
                            nc.vector.tensor_copy(ut[:, :], tp[:, :])
                            vp = psum2.tile([128, 128], dt.float32, tag="vp")
                            nc.tensor.matmul(vp[:, :], ut[:, :], W2_sb[:, :],
                                             start=True, stop=True)
                            nc.vector.tensor_copy(
                                out_wide[:, j0 + jj:j0 + jj + 2, :].rearrange(
                                    "p w h -> p (w h)"), vp[:, :])
                        nc.sync.dma_start(
                            wout[:, :].rearrange("(j l) h -> l j h", l=128)
                            [:, j0:j0 + nw, :], out_wide[:, j0:j0 + nw, :])
                    else:
                        zw = work.tile([128, MAXW, 2 * H2], dt.float32, tag="zw")
                        nc.vector.memset(zw[:, :nw, :], 0.0)
                        nc.vector.tensor_copy(zw[:, :nw, :H2], acc[:, :nw, :])
                        nc.sync.dma_start(
                            wout[:, :].rearrange("(j l) h -> l j h", l=128)
                            [:, j0:j0 + nw, :], zw[:, :nw, :])

            layer(w_full, H1, dinv64_d, b1_sb, True, w_loc, v_loc, v_in)
            nc.gpsimd.collective_compute(
                "AllGather", OP.bypass, replica_groups=rg,
                ins=[v_in.ap().opt()], outs=[v_full.ap().opt()])

            layer(v_full, H2, dinv32_d, b2_sb, False, v_loc, None, z_in)
            nc.gpsimd.collective_compute(
                "AllGather", OP.bypass, replica_groups=rg,
                ins=[z_in.ap().opt()], outs=[z_full.ap().opt()])

            # ---- decode
            logit_sb = pers.tile([128, S_EV], dt.float32)
            for g in range(16):
                ns = int(gslots[g])
                if ns == 0:
                    continue
                gb = int(gslots[:g].sum())
                r0, r1 = g // NREG, g % NREG
                za = msgp.tile([128, MAXEV, 64], dt.float32, tag="za")
                zb = msgp.tile([128, MAXEV, 64], dt.float32, tag="zb")
                ia = io.tile([128, 8 * MAXEV], dt.int16, tag="ia")
                ib = io.tile([128, 8 * MAXEV], dt.int16, tag="ib")
                nc.sync.dma_start(ia[:, :ns * 8], ev0_d[:, gb * 8:(gb + ns) * 8])
                nc.sync.dma_start(ib[:, :ns * 8], ev1_d[:, gb * 8:(gb + ns) * 8])
                for t0 in range(0, ns, 8):
                    ts = min(8, ns - t0)
                    n = ts * 128
                    nc.gpsimd.dma_gather(
                        out_ap=za[:, t0:t0 + ts, :],
                        in_ap=z_full[r0 * REG:r0 * REG + REGION_ROWS[r0], :],
                        idxs_ap=ia[:, t0 * 8:(t0 + ts) * 8], num_idxs=n,
                        num_idxs_reg=n, elem_size=64, single_packet=True,
                        queue_num=nextq())
                    nc.gpsimd.dma_gather(
                        out_ap=zb[:, t0:t0 + ts, :],
                        in_ap=z_full[r1 * REG:r1 * REG + REGION_ROWS[r1], :],
                        idxs_ap=ib[:, t0 * 8:(t0 + ts) * 8], num_idxs=n,
                        num_idxs_reg=n, elem_size=64, single_packet=True,
                        queue_num=nextq())
                nc.vector.tensor_tensor(za[:, :ns, :H2], za[:, :ns, :H2],
                                        zb[:, :ns, :H2], op=OP.mult)
                nc.vector.tensor_reduce(
                    logit_sb[:, gb:gb + ns], za[:, :ns, :H2], axis=AT.X,
                    op=OP.add)
            nc.sync.dma_start(out_d[:, :], logit_sb[:, :])

    nc.compile()
    return nc


# ---------------------------------------------------------------- entry
_CACHE = {}


def kernel(x, W1, b1, W2, b2, edge_index, pos_edge_index, neg_edge_index,
           _profile=False):
    from concourse.bass_utils import run_bass_kernel_spmd

    if "k" not in _CACHE:
        meta = _preprocess(edge_index, pos_edge_index, neg_edge_index)
        nc = _build(meta)
        _CACHE["k"] = (meta, nc)
    meta, nc = _CACHE["k"]
    in_maps = _stage_inputs(x, W1, b1, W2, b2, meta)

    trace = False
    if _profile:
        import sys
        import types
        import concourse.bass_utils as bass_utils
        import antenv
        from trn_agent_boot.trn_boot import _ntff_profile_via_ctypes
        hook = _ntff_profile_via_ctypes("/opt/axon/libaxon_pjrt.so")
        mod = types.ModuleType("antenv.axon_hooks")
        mod.get_axon_ntff_profile_hook = lambda: hook
        mod.set_axon_ntff_profile_hook = lambda h: None
        sys.modules["antenv.axon_hooks"] = mod
        antenv.axon_hooks = mod
        bass_utils.upload_artifacts = lambda tmpdir: tmpdir
        trace = True

    res = run_bass_kernel_spmd(nc, in_maps, core_ids=list(range(NCORES)),
                               trace=trace)
    logits = np.empty(2 * N_EVAL, np.float32)
    allout = np.stack([res.results[c]["logits"] for c in range(NCORES)])
    lp = meta["ev_lpos"]
    logits[meta["ev_edge"]] = allout[meta["ev_core"], lp % 128, lp // 128]
    if _profile:
        return logits, res.exec_time_ns
    return logits
